# revision 2
# baseline (speedup 1.0000x reference)
"""CARAFE++ content-aware upsampling kernel for Trainium2 (8 NeuronCores), v2.

Problem: x (4, 256, 64, 64) f32; 1x1 compress conv (256->64) + relu;
3x3 encoder conv (64->100); softmax over 25 taps; content-aware reassembly
(5x5 dynamic per-pixel filter, scale 2); flat pixel rearrangement to
(4, 256, 128, 128).

Sharding: 8 cores = 4 batches x 2 row-halves (32 rows each + halo).
All compute per-core independent (no collectives).

v2 changes vs baseline:
  - host sends x pre-cast fp16 in BOTH channel-major (conv1) and pixel-major
    (reassembly lhsT) layouts; no on-device cast / x transposes
  - conv2 packs tap rows 0+1 per column into K=128 matmuls via a
    row-duplicated +1-row-shifted feat copy (featd[64:128]): 6 matmuls/tile
  - ALL transposes (wk, sums, scatter output panels) moved off the PE onto
    the DMA XBAR (dma_start_transpose, panel-major 3D fold)
  - output stored fp16, upcast on host
"""
import sys

sys.path.insert(0, "/opt/trn_rl_repo")

import numpy as np
from contextlib import ExitStack

import concourse.bass as bass
import concourse.bacc as bacc
import concourse.tile as tile
from concourse.tile import add_dep_helper
from concourse import mybir
from concourse.bass_utils import run_bass_kernel_spmd

B, C, H, W = 4, 256, 64, 64
SCALE, K, COMP, G = 2, 5, 4, 1
MID = 64
ENC = 100          # K*K*SCALE*SCALE
NROW = 36          # x rows per core (32 + 2 halo each side)
NPX = NROW * W     # 2304
FROW = 34          # feat rows r0-1 .. r0+32
FPW = W + 2        # 66, feat row W-padded
NBLK = 16          # output row-pair blocks per core
NJB = 18           # x row-pair panels per core

f32 = mybir.dt.float32
f16 = mybir.dt.float16
i16 = mybir.dt.int16

_CACHE = {}


def _build_idxs():
    """Per-partition scatter indices. Partition = out-center px (rt, w); slot
    = (p, dy, dx) wk channel order; dest = (dj*4+p)*128 + rb*64 + wi so the
    XBAR panel-major transpose yields S panels grouped (dj, p)."""
    idxs = np.full((128, 100), -1, np.int16)
    for rt in range(2):
        for w in range(W):
            part = rt * W + w
            for p in range(4):
                for dy in range(-2, 3):
                    dj = (rt + dy + 2) // 2
                    rb = (rt + dy) % 2
                    for dx in range(-2, 3):
                        wi = w + dx
                        if 0 <= wi < W:
                            slot = p * 25 + (dy + 2) * 5 + (dx + 2)
                            idxs[part, slot] = (dj * 4 + p) * 128 + rb * 64 + wi
    return idxs


def _build_nc():
    nc = bacc.Bacc("TRN2", target_bir_lowering=False, debug=False, num_devices=8)

    # ---- DRAM I/O (per-core shapes)
    d_x = nc.dram_tensor("x", [C, NPX], f16, kind="ExternalInput")
    d_xt = nc.dram_tensor("xt", [128, NJB * 2 * 128], f16, kind="ExternalInput")
    d_wc = nc.dram_tensor("wc", [C, MID], f16, kind="ExternalInput")      # W_comp.T
    d_wep = nc.dram_tensor("wep", [128, 3 * ENC], f16, kind="ExternalInput")
    d_wes = nc.dram_tensor("wes", [MID, 3 * ENC], f16, kind="ExternalInput")
    d_bc = nc.dram_tensor("bc", [MID, 1], f32, kind="ExternalInput")
    d_be = nc.dram_tensor("be", [ENC, 1], f32, kind="ExternalInput")
    d_ones = nc.dram_tensor("ones", [ENC, 4], f16, kind="ExternalInput")
    d_idx = nc.dram_tensor("idx", [128, 100], i16, kind="ExternalInput")
    d_out = nc.dram_tensor("out", [C, 32 * 256], f16, kind="ExternalOutput")

    with tile.TileContext(nc) as tc, ExitStack() as ctx:
        sb1 = ctx.enter_context(tc.tile_pool(name="sb1", bufs=1))
        sbw = ctx.enter_context(tc.tile_pool(name="sbw", bufs=2))
        ps = ctx.enter_context(tc.tile_pool(name="ps", bufs=1, space="PSUM"))

        # ---- load weights / constants (sync ring)
        wc0 = sb1.tile([128, MID], f16, tag="wc0")
        wc1 = sb1.tile([128, MID], f16, tag="wc1")
        nc.scalar.dma_start(out=wc0, in_=d_wc[0:128, :])
        nc.scalar.dma_start(out=wc1, in_=d_wc[128:256, :])
        wep = sb1.tile([128, 3, ENC], f16, tag="wep")
        wes = sb1.tile([MID, 3, ENC], f16, tag="wes")
        nc.scalar.dma_start(out=wep, in_=d_wep[:].rearrange("m (t o) -> m t o", t=3))
        nc.scalar.dma_start(out=wes, in_=d_wes[:].rearrange("m (t o) -> m t o", t=3))
        bc = sb1.tile([MID, 1], f32, tag="bc")
        be = sb1.tile([ENC, 1], f32, tag="be")
        nc.scalar.dma_start(out=bc, in_=d_bc[:])
        nc.scalar.dma_start(out=be, in_=d_be[:])
        ones = sb1.tile([ENC, 4], f16, tag="ones")
        nc.scalar.dma_start(out=ones, in_=d_ones[:])
        sidx = sb1.tile([128, 100], i16, tag="sidx")
        nc.scalar.dma_start(out=sidx, in_=d_idx[:])

        # warm the Exp activation table during load shadow
        scratch = sb1.tile([ENC, 2], f32, tag="scratch")
        nc.scalar.activation(out=scratch[:, 0:1], in_=be[:],
                             func=mybir.ActivationFunctionType.Exp,
                             bias=be[:], scale=1.0)

        # ---- load x (fp16, both layouts; chunked for early conv start)
        x0 = sb1.tile([128, NPX], f16, tag="x0")
        x1 = sb1.tile([128, NPX], f16, tag="x1")
        nc.scalar.dma_start(out=x0[:, 0:1152], in_=d_x[0:128, 0:1152])
        nc.scalar.dma_start(out=x1[:, 0:1152], in_=d_x[128:256, 0:1152])
        nc.scalar.dma_start(out=x0[:, 1152:NPX], in_=d_x[0:128, 1152:NPX])
        nc.scalar.dma_start(out=x1[:, 1152:NPX], in_=d_x[128:256, 1152:NPX])
        xt = sb1.tile([128, NJB, 2, 128], f16, tag="xt")
        nc.sync.dma_start(
            out=xt, in_=d_xt[:].rearrange("q (j c m) -> q j c m", j=NJB, c=2))

        featd = sb1.tile([128, FROW * FPW], f16, tag="featd")
        nc.vector.memset(featd, 0.0)

        # ---- conv1 (1x1, 256->64) + relu -> featd halves (fp16, W-padded)
        xs = [x0, x1]
        for nt in range(5):
            n0 = W + nt * 512          # px offset into x
            n = min(512, 2240 - n0)
            pf = ps.tile([MID, 512], f32, tag="pf", bufs=2)
            nc.tensor.matmul(pf[:, :n], wc0[:], x0[:, n0:n0 + n],
                             start=True, stop=False)
            nc.tensor.matmul(pf[:, :n], wc1[:], x1[:, n0:n0 + n],
                             start=False, stop=True)
            fp0 = n0 // W - 1
            nr = n // W
            src = pf[:, :n].rearrange("m (r w) -> m r w", w=W)
            fd1h = featd[0:64]
            dst1 = bass.AP(
                tensor=fd1h.tensor, offset=fd1h.offset + fp0 * FPW + 1,
                ap=[fd1h.ap[0], [FPW, nr], [1, W]],
            )
            nc.scalar.activation(out=dst1, in_=src,
                                 func=mybir.ActivationFunctionType.Relu,
                                 bias=bc[:], scale=1.0)
            # featd2: partitions 64-127 hold feat shifted one row up
            fd2 = featd[64:128]
            if fp0 == 0:
                src2 = bass.AP(
                    tensor=pf.tensor, offset=pf.offset + W,
                    ap=[pf.ap[0], [W, nr - 1], [1, W]],
                )
                dst2 = bass.AP(
                    tensor=fd2.tensor, offset=fd2.offset + 1,
                    ap=[fd2.ap[0], [FPW, nr - 1], [1, W]],
                )
            else:
                src2 = bass.AP(
                    tensor=pf.tensor, offset=pf.offset,
                    ap=[pf.ap[0], [W, nr], [1, W]],
                )
                dst2 = bass.AP(
                    tensor=fd2.tensor,
                    offset=fd2.offset + (fp0 - 1) * FPW + 1,
                    ap=[fd2.ap[0], [FPW, nr], [1, W]],
                )
            nc.scalar.activation(out=dst2, in_=src2,
                                 func=mybir.ActivationFunctionType.Relu,
                                 bias=bc[:], scale=1.0)

        # ---- conv2 (3x3, 64->100, row-pair packed) + exp; sums; XBAR wkT
        wk = sb1.tile([112, 2048], f16, tag="wk")
        sums16 = sb1.tile([16, 2048], f16, tag="sums16")
        wkT = sb1.tile([128, NBLK, 112], f16, tag="wkT")
        sumsT = sb1.tile([128, NBLK, 16], f16, tag="sumsT")
        nc.vector.memset(wk[96:112, :], 0.0)
        nc.vector.memset(sums16[:, :], 0.0)
        wkT_x, sumsT_x = [], []
        for nt in range(4):
            h0 = nt * 8
            pw = ps.tile([ENC, 512], f32, tag="pw", bufs=2)
            for j in range(3):
                rhs = bass.AP(
                    tensor=featd.tensor, offset=featd.offset + h0 * FPW + j,
                    ap=[featd.ap[0], [FPW, 8], [1, W]],
                )
                nc.tensor.matmul(pw[:], wep[:, j, :], rhs,
                                 start=(j == 0), stop=False)
            fd1 = featd[0:64]
            for j in range(3):
                rhs = bass.AP(
                    tensor=fd1.tensor,
                    offset=fd1.offset + (h0 + 2) * FPW + j,
                    ap=[fd1.ap[0], [FPW, 8], [1, W]],
                )
                nc.tensor.matmul(pw[:], wes[:, j, :], rhs,
                                 start=False, stop=(j == 2))
            exp_i = nc.scalar.activation(
                out=wk[0:ENC, nt * 512:(nt + 1) * 512], in_=pw[:],
                func=mybir.ActivationFunctionType.Exp, bias=be[:], scale=1.0)
            psm = ps.tile([4, 512], f32, tag="psm", bufs=1)
            nc.tensor.matmul(psm[:], ones[:], wk[0:ENC, nt * 512:(nt + 1) * 512],
                             start=True, stop=True)
            sum_i = nc.scalar.activation(
                out=sums16[0:4, nt * 512:(nt + 1) * 512], in_=psm[:],
                func=mybir.ActivationFunctionType.Copy, scale=1.0)
            # XBAR transposes are not dep-tracked by Tile: add deps manually
            wx = nc.sync.dma_start_transpose(
                out=wkT[:, nt * 4:(nt + 1) * 4, :],
                in_=wk[:, nt * 512:(nt + 1) * 512])
            add_dep_helper(wx.ins, exp_i.ins, reason="xbar wkT reads wk")
            sx = nc.sync.dma_start_transpose(
                out=sumsT[:, nt * 4:(nt + 1) * 4, :],
                in_=sums16[:, nt * 512:(nt + 1) * 512])
            add_dep_helper(sx.ins, sum_i.ins, reason="xbar sumsT reads sums16")
            wkT_x.append(wx)
            sumsT_x.append(sx)

        # ---- per-block: normalize, scatter, XBAR-transpose, reassemble
        osegs = [None, None]
        sdst_reader = [None, None]   # last XBAR read of each sdst ring slot
        last_mm = [None, None]       # last PE read of each T ring slot
        for t in range(NBLK):
            recipT = sbw.tile([128, 4], f32, tag="recipT", bufs=2)
            rec_i = nc.vector.reciprocal(recipT[:], sumsT[:, t, 0:4])
            add_dep_helper(rec_i.ins, sumsT_x[t // 4].ins,
                           reason="recip reads xbar sumsT")
            wkT16 = sbw.tile([128, 100], f16, tag="wkT16", bufs=3)
            rb = bass.AP(tensor=recipT.tensor, offset=recipT.offset,
                         ap=[recipT.ap[0], [1, 4], [0, 25]])
            mul_i = nc.vector.tensor_mul(
                wkT16[:].rearrange("q (p k) -> q p k", k=25),
                wkT[:, t, 0:100].rearrange("q (p k) -> q p k", k=25),
                rb,
            )
            add_dep_helper(mul_i.ins, wkT_x[t // 4].ins,
                           reason="normalize reads xbar wkT")
            sdst = sbw.tile([128, 1536], f16, tag="sdst", bufs=2)
            scat_i = nc.gpsimd.local_scatter(
                out_ap=sdst[:], data_ap=wkT16[:], idxs_ap=sidx[:],
                channels=128, num_elems=1536, num_idxs=100,
            )
            if sdst_reader[t % 2] is not None:
                add_dep_helper(scat_i.ins, sdst_reader[t % 2].ins,
                               reason="WAR: scatter overwrites xbar-read sdst")
            T = sbw.tile([128, 12, 128], f16, tag="T", bufs=2)
            tx = nc.sync.dma_start_transpose(out=T[:], in_=sdst[:])
            add_dep_helper(tx.ins, scat_i.ins, reason="xbar T reads sdst")
            if last_mm[t % 2] is not None:
                add_dep_helper(tx.ins, last_mm[t % 2].ins,
                               reason="WAR: xbar overwrites PE-read T")
            sdst_reader[t % 2] = tx

            if t % 2 == 0:
                osegs[0] = sbw.tile([128, 1024], f16, tag="oseg0", bufs=2,
                                    name="oseg0")
                osegs[1] = sbw.tile([128, 1024], f16, tag="oseg1", bufs=2,
                                    name="oseg1")
            for ch in range(2):
                po = ps.tile([128, 512], f32, tag="po", bufs=3)
                for dj in range(3):
                    mm_i = nc.tensor.matmul(
                        po[:], xt[:, t + dj, ch, :], T[:, dj * 4:(dj + 1) * 4, :],
                        start=(dj == 0), stop=(dj == 2),
                    )
                    if ch == 0 and dj == 0:
                        add_dep_helper(mm_i.ins, tx.ins,
                                       reason="PE reads xbar T")
                last_mm[t % 2] = mm_i
                # evict with (p, rt, w) -> (rt, w, p) interleave, cast fp16
                dst = bass.AP(
                    tensor=osegs[ch].tensor,
                    offset=osegs[ch].offset + (t % 2) * 512,
                    ap=[osegs[ch].ap[0], [256, 2], [4, 64], [1, 4]],
                )
                src = bass.AP(tensor=po.tensor, offset=po.offset,
                              ap=[po.ap[0], [64, 2], [1, 64], [128, 4]])
                if ch == 0:
                    nc.scalar.activation(out=dst, in_=src,
                                         func=mybir.ActivationFunctionType.Copy,
                                         scale=1.0)
                else:
                    nc.vector.tensor_copy(dst, src)
            if t % 2 == 1:
                for ch in range(2):
                    nc.scalar.dma_start(
                        out=d_out[ch * 128:(ch + 1) * 128,
                                  (t - 1) * 512:(t + 1) * 512],
                        in_=osegs[ch][:],
                    )

    nc.compile()
    return nc


def _host_prep(x, W_comp, b_comp, W_enc, b_enc):
    """Build per-core input maps (layout/dtype prep only)."""
    idxs = _build_idxs()
    wcT = np.ascontiguousarray(W_comp.T).astype(np.float16)            # (256, 64)
    wep = np.zeros((128, 3 * ENC), np.float16)
    wes = np.zeros((MID, 3 * ENC), np.float16)
    for j in range(3):
        wep[0:64, j * ENC:(j + 1) * ENC] = W_enc[:, :, 0, j].T
        wep[64:128, j * ENC:(j + 1) * ENC] = W_enc[:, :, 1, j].T
        wes[:, j * ENC:(j + 1) * ENC] = W_enc[:, :, 2, j].T
    bc = np.ascontiguousarray(b_comp.reshape(MID, 1)).astype(np.float32)
    be = np.ascontiguousarray(b_enc.reshape(ENC, 1)).astype(np.float32)
    ones = np.zeros((ENC, 4), np.float16)
    for p in range(4):
        ones[p * 25:(p + 1) * 25, p] = 1.0

    xp = np.pad(x, ((0, 0), (0, 0), (2, 2), (0, 0)))   # (B, C, 68, 64)
    in_maps = []
    for core in range(8):
        b, half = core // 2, core % 2
        r0 = 32 * half
        xs = np.ascontiguousarray(
            xp[b, :, r0:r0 + NROW, :].reshape(C, NPX)).astype(np.float16)
        xtc = np.ascontiguousarray(
            xs.reshape(2, 128, NJB, 128).transpose(3, 2, 0, 1)
        ).reshape(128, NJB * 2 * 128)
        in_maps.append(dict(x=xs, xt=xtc, wc=wcT, wep=wep, wes=wes,
                            bc=bc, be=be, ones=ones, idx=idxs))
    return in_maps


def kernel(x, W_comp, b_comp, W_enc, b_enc):
    x = np.asarray(x, np.float32)
    W_comp = np.asarray(W_comp, np.float32)
    b_comp = np.asarray(b_comp, np.float32)
    W_enc = np.asarray(W_enc, np.float32)
    b_enc = np.asarray(b_enc, np.float32)

    if "nc" not in _CACHE:
        _CACHE["nc"] = _build_nc()
    nc = _CACHE["nc"]

    in_maps = _host_prep(x, W_comp, b_comp, W_enc, b_enc)
    res = run_bass_kernel_spmd(nc, in_maps, core_ids=list(range(8)))

    out = np.empty((B, C, 128, 128), np.float32)
    for core in range(8):
        b, half = core // 2, core % 2
        seg = res.results[core]["out"].astype(np.float32)   # (256, 8192) f16
        out[b, :, 64 * half:64 * (half + 1), :] = seg.reshape(C, 64, 128)
    return out


if __name__ == "__main__":
    d = np.load("/tmp/carafe_ref.npz")
    expected = d["expected"]
    out = kernel(**{k: d[k] for k in ["x", "W_comp", "b_comp", "W_enc", "b_enc"]})
    err = np.abs(out - expected)
    scale = np.abs(expected).max()
    print(f"absmax err: {err.max():.4e}  rel: {err.max()/scale:.4e}")
